# revision 14
# baseline (speedup 1.0000x reference)
# Multi-head self-attention with RoPE on 8 Trainium2 NeuronCores.
#
# Sharding: batch x head-group. Core c handles batch b = c//4 and heads
# 4*(c%4) .. 4*(c%4)+3 (4 of 16 heads). Each core computes Q/K/V
# projections for its heads from the full (transposed) x[b], runs
# attention, and produces a partial output projection
# Y_partial = O_core^T.T @ Wo[rows-of-its-heads]. The host sums the four
# partials per batch and adds the (constant) bias terms.
#
# Matmul dtypes: projections and S = Q.K^T run in float32r; the
# post-softmax path (P, V, O, Wo) runs in bf16 with fp32 PSUM accumulation.
#
# Schedule (the point of this rewrite): the PE runs one dense in-order
# stream, software-pipelined so it never waits on the Act engine's exp
# round-trip. Attention is 16 rounds per (pair, q-tile) block; round r
# emits S(r) into a 4-bank PSUM ring (2 banks per round, 2 rounds in
# flight) and PV(r-2) consuming the exp'd P of two rounds ago. Blocks run
# pr-major ((pr0,qi0..3) then (pr1,qi0..3)); the pair-1 Q/K projections
# and the Wo matmuls are queued as PE "filler" popped one closure per
# round, keeping the PE stream dense enough that it holds its top p-state
# and the Act engine (exp is ~127us/core, PE ~164us/core) never gates it.
#
# PSUM: ring 4 banks + O-accumulators 2 (65 rows: 64 d + Z) + proj/Y 2.
# Engines: exp on Act only; evictions/V-copy/broadcast/Y-evict on Pool;
# rope shuffle+sin-mul+add and division recip/mult on DVE; cos-mul on
# Pool. Input DMAs ride the SP ring (wq/wk, x-lo, wo) and Act ring
# (wv/tables, x-hi); output DMAs go through Pool's SWDGE queue so the
# HWDGE rings carry only inputs and next-iteration transfers complete
# mid-iteration instead of stalling the loop boundary.
#
# RoPE: head-dim rows are pair-interleaved (d' = [0,32,1,33,...]) via a host
# permutation of Wq/Wk columns so the rotate-half partner lives on the
# adjacent partition; a DVE stream_shuffle (pair swap) + 2 muls + 1 add
# apply the rotation with band-replicated, sign-baked cos/sin tables.

import os
import sys

import numpy as np

try:
    import ml_dtypes

    BF16 = np.dtype(ml_dtypes.bfloat16)
except ImportError:  # pragma: no cover
    BF16 = None

for _p in ("/opt/trn_rl_repo", os.path.expanduser("~/.axon_site/_ro/trn_rl_repo")):
    if os.path.isdir(_p) and _p not in sys.path:
        sys.path.insert(0, _p)

B, T, D = 2, 2048, 1024
NHEADS, HD, HALF = 16, 64, 32
HPC = 4  # heads per core
N_CORES = 8
ROPE_BASE = 10000.0
SCALE = float(HD) ** -0.5  # 0.125
NDC = D // 128  # 8 contraction chunks for the projections
NKC = T // 128  # 16 k chunks per head

_SHUF_MASK = [i ^ 1 for i in range(32)]

_ctx: dict = {}


def _build_nc(iters: int = 0, phase: str = "full"):
    import concourse.bacc as bacc
    import concourse.mybir as mybir
    import concourse.tile as tile

    f32 = mybir.dt.float32
    f32r = mybir.dt.float32r
    bf16 = mybir.dt.bfloat16
    u32 = mybir.dt.uint32
    Exp = mybir.ActivationFunctionType.Exp
    MUL = mybir.AluOpType.mult
    ADD = mybir.AluOpType.add

    nc = bacc.Bacc("TRN2", target_bir_lowering=False, debug=False)

    # packed inputs (see _host_inputs for layouts)
    xt_d = nc.dram_tensor("xtp", [128, NDC * T], f32, kind="ExternalInput").ap()
    wp1_d = nc.dram_tensor("wp1", [128, 4096], f32, kind="ExternalInput").ap()
    wp2_d = nc.dram_tensor("wp2", [128, 6148], f32, kind="ExternalInput").ap()
    wo_d = nc.dram_tensor("wop", [128, 2048], bf16, kind="ExternalInput").ap()
    y_d = nc.dram_tensor("y", [T, D], f32, kind="ExternalOutput").ap()
    y_r = y_d.rearrange("(b p) e -> p b e", p=128)  # [128, 16, 1024]

    with tile.TileContext(nc) as tc:
        with (
            tc.tile_pool(name="xtpool", bufs=2) as xtpool,
            tc.tile_pool(name="wpool", bufs=1) as wpool,
            tc.tile_pool(name="wopool", bufs=2) as wopool,
            tc.tile_pool(name="qkpool", bufs=4) as qkpool,
            tc.tile_pool(name="ppool", bufs=4) as ppool,
            tc.tile_pool(name="shpool", bufs=2) as shpool,
            tc.tile_pool(name="otpool", bufs=2) as otpool,
            tc.tile_pool(name="vpool", bufs=16) as vpool,
            tc.tile_pool(name="small", bufs=2) as small,
            tc.tile_pool(name="ypool", bufs=2) as ypool,
            tc.tile_pool(name="psum", bufs=1, space="PSUM") as psum,
        ):

            def body():
                # ---- input DMAs: SP ring + Act ring (inputs only) ----
                wp1 = wpool.tile([128, 4096], f32r, tag="wp1", name="wp1")
                nc.sync.dma_start(out=wp1[:], in_=wp1_d.bitcast(f32r))
                xta = xtpool.tile([128, 4 * T], f32r, tag="xt", name="xta")
                nc.sync.dma_start(out=xta[:], in_=xt_d[:, 0 : 4 * T].bitcast(f32r))
                wo_t = wopool.tile([128, 2048], bf16, tag="wo", name="wo_t")
                nc.sync.dma_start(out=wo_t[:], in_=wo_d)
                wp2 = wpool.tile([128, 6148], f32r, tag="wp2", name="wp2")
                nc.scalar.dma_start(out=wp2[:], in_=wp2_d.bitcast(f32r))
                xtb = xtpool.tile([128, 4 * T], f32r, tag="xt", name="xtb")
                nc.scalar.dma_start(out=xtb[:], in_=xt_d[:, 4 * T :].bitcast(f32r))

                def xt_sl(ch, lo, hi):
                    t_ = xta if ch < 4 else xtb
                    return t_[:, (ch % 4) * T + lo : (ch % 4) * T + hi]

                def wq_sl(ch, pr):  # [128 Dchunk, 128]
                    c0 = ch * 256 + pr * 128
                    return wp1[:, c0 : c0 + 128]

                def wk_sl(ch, pr):
                    c0 = 2048 + ch * 256 + pr * 128
                    return wp1[:, c0 : c0 + 128]

                def wv_sl(ch):  # [128, 256]
                    return wp2[:, ch * 256 : (ch + 1) * 256]

                cos_t = wp2[:, 2048:4096]
                sin_t = wp2[:, 4096:6144]
                qb_t = wp2.bitcast(f32)[:, 6144:6146]
                kb_t = wp2.bitcast(f32)[:, 6146:6148]

                # 4-bank S ring: S bank c uses cols (c%4)*512 .. +512.
                # Each bank holds both heads of one k-chunk (256 q-cols per
                # head); exp units cover 2 banks ([128,1024]); with 256-q
                # tiles each bank carries ~1.1us of PE work, so a 2-unit ring
                # already hides the exp round-trip latency.
                ring = psum.tile([128, 2048], f32, tag="ring", name="ring")

                qts = [None, None]
                kts = [None, None]
                vts = []
                ot0 = otpool.tile([128, T], bf16, tag="ot", name="ot0")
                ot1 = otpool.tile([128, T], bf16, tag="ot", name="ot1")
                ots = [ot0, ot1]

                def rope_slice(t_, ts, name):
                    # shuffle+sin-mul on DVE; cos-mul+add on Pool (SBUF only)
                    sl = slice(ts * 512, (ts + 1) * 512)
                    sh = shpool.tile([128, 512], f32r, tag="sh", name=f"sh{name}")
                    nc.vector.stream_shuffle(
                        sh.bitcast(u32)[:], t_.bitcast(u32)[:, sl], _SHUF_MASK
                    )
                    nc.vector.tensor_tensor(
                        out=t_[:, sl], in0=t_[:, sl], in1=cos_t[:, sl], op=MUL
                    )
                    nc.vector.tensor_tensor(
                        out=sh[:], in0=sh[:], in1=sin_t[:, sl], op=MUL
                    )
                    nc.vector.tensor_tensor(
                        out=t_[:, sl], in0=t_[:, sl], in1=sh[:], op=ADD
                    )

                def qk_half(qt, w_sl, bias_t, pr, ts, half, ps_box, name, ps=None):
                    # half (4 D-chunks) of a 512-col Q/K projection t-block;
                    # second half evicts (+bias, DVE: PSUM read) + rope.
                    # ps: explicit PSUM AP (ring scratch) or pool-allocated.
                    if half == 0:
                        ps_box[0] = (
                            ps
                            if ps is not None
                            else psum.tile(
                                [128, 512], f32, tag="py", bufs=2, name=f"ps{name}"
                            )
                        )
                    ps = ps_box[0]
                    for ch in range(half * 4, half * 4 + 4):
                        nc.tensor.matmul(
                            ps[:],
                            w_sl(ch, pr),
                            xt_sl(ch, ts * 512, (ts + 1) * 512),
                            start=(ch == 0),
                            stop=(ch == NDC - 1),
                        )
                    if half == 1:
                        nc.vector.tensor_scalar_add(
                            qt[:, ts * 512 : (ts + 1) * 512],
                            ps[:],
                            bias_t[:, pr : pr + 1],
                        )
                        rope_slice(qt, ts, name)

                def qk_group(qt, w_sl, bias_t, pr, ts, name, ps=None):
                    box = [None]
                    qk_half(qt, w_sl, bias_t, pr, ts, 0, box, name, ps=ps)
                    qk_half(qt, w_sl, bias_t, pr, ts, 1, box, name)

                def v_group(tk):
                    vt = vpool.tile([128, HPC * 65], bf16, tag="v", name=f"v{tk}")
                    nc.vector.memset(
                        vt.rearrange("p (h c) -> p h c", c=65)[:, :, 64:65], 1.0
                    )
                    ps = psum.tile([128, 256], f32, tag="py", bufs=2, name=f"psv{tk}")
                    for ch in range(NDC):
                        nc.tensor.matmul(
                            ps[:],
                            xt_sl(ch, tk * 128, (tk + 1) * 128),
                            wv_sl(ch),
                            start=(ch == 0),
                            stop=(ch == NDC - 1),
                        )
                    nc.vector.tensor_copy(
                        vt.rearrange("p (h c) -> p h c", c=65)[:, :, 0:64],
                        ps.rearrange("p (h c) -> p h c", c=64),
                    )
                    vts.append(vt)

                # ---- projection phase (straight-line): just enough to start
                # attention. PSUMs go through the still-idle ring banks so
                # there is no ping-pong WAR stall; everything else is filler.
                qts[0] = qkpool.tile([128, T], f32r, tag="qk", name="qt0")
                kts[0] = qkpool.tile([128, T], f32r, tag="qk", name="kt0")
                qts[1] = qkpool.tile([128, T], f32r, tag="qk", name="qt1")
                kts[1] = qkpool.tile([128, T], f32r, tag="qk", name="kt1")
                qk_group(qts[0], wq_sl, qb_t, 0, 0, "q00", ps=ring[:, 0:512])
                for ts in range(3):
                    qk_group(
                        kts[0], wk_sl, kb_t, 0, ts, f"k0{ts}",
                        ps=ring[:, (ts + 1) * 512 : (ts + 2) * 512],
                    )
                qk_group(kts[0], wk_sl, kb_t, 0, 3, "k03")

                # ---- PE filler queues ----
                # vfill: V-projection groups, popped on a fixed schedule in
                # the first attention block (V[kc] must precede PV of kc).
                vfill = [lambda tk=tk: v_group(tk) for tk in range(NKC)]
                # normal queue: (pe_ns_estimate, closure), debt-paced.
                fillers = []
                debt = [0.0]

                def pop_filler():
                    debt[0] += 400.0
                    while fillers and debt[0] >= fillers[0][0]:
                        cost, fn = fillers.pop(0)
                        debt[0] -= cost
                        fn()
                    if not fillers:
                        debt[0] = 0.0

                # remaining projections as filler, deadline order: qt0 t-blocks
                # 1-3 (pr0 blocks 2-7), then pair-1 Q/K (pr1 phase).
                for qt, w_sl_, b_t, pr_, ts, nm in (
                    [(qts[0], wq_sl, qb_t, 0, t, "q0") for t in range(1, 4)]
                    + [(qts[1], wq_sl, qb_t, 1, 0, "q1")]
                    + [(kts[1], wk_sl, kb_t, 1, t, "k1") for t in range(4)]
                    + [(qts[1], wq_sl, qb_t, 1, t, "q1") for t in range(1, 4)]
                ):
                    box = [None]
                    for half in range(2):
                        fillers.append(
                            (
                                900.0,
                                lambda qt=qt, w=w_sl_, b=b_t, p=pr_, ts=ts, h=half, bx=box, nm=nm: qk_half(
                                    qt, w, b, p, ts, h, bx, f"{nm}{ts}"
                                ),
                            )
                        )

                def queue_y(qy):
                    # output projection for 512-wide q-tile qy (both pairs)
                    for yt in range(2):
                        ysb = ypool.tile([128, 2048], f32, tag="y", name=f"y{qy}{yt}")

                        def grp(sub, eh, ysb=ysb, yt=yt):
                            tt = 4 * qy + 2 * yt + sub
                            yps = psum.tile(
                                [128, 512], f32, tag="py", bufs=2, name=f"yp{tt}{eh}"
                            )
                            for r in range(2):
                                nc.tensor.matmul(
                                    yps[:],
                                    ots[r][:, tt * 128 : (tt + 1) * 128],
                                    wo_t[
                                        :, r * 1024 + eh * 512 : r * 1024 + (eh + 1) * 512
                                    ],
                                    start=(r == 0),
                                    stop=(r == 1),
                                )
                            nc.vector.tensor_copy(
                                ysb[:, sub * 1024 + eh * 512 : sub * 1024 + (eh + 1) * 512],
                                yps[:],
                            )

                        for sub in range(2):
                            for eh in range(2):
                                fillers.append(
                                    (550.0, lambda s=sub, e=eh, g=grp: g(s, e))
                                )

                        def ydma(ysb=ysb, yt=yt):
                            nc.scalar.dma_start(
                                out=y_r[:, 4 * qy + 2 * yt : 4 * qy + 2 * yt + 2, :],
                                in_=ysb.rearrange("p (b e) -> p b e", e=1024),
                            )

                        fillers.append((800.0, ydma))

                # ---- attention: pr-major blocks of 512-q tiles ----
                # Per block: 16 k-chunk slots; slot c emits S(c) (two
                # full-bank matmuls, one per head) and PV(c-4); exp after each
                # S pair covers the 2 banks. S runs 4 banks ahead of the WAR.
                gb = 0  # global bank counter (ring index)
                NQB = 4  # 512-wide q-tiles per pair
                for pr in range(2):
                    for qi in range(NQB):
                        qs = qi * 512
                        oaccs = [
                            psum.tile(
                                [65, 512], f32, tag="oa", bufs=2, name=f"oa{pr}{qi}{h}"
                            )
                            for h in range(2)
                        ]
                        pts = [None] * NKC

                        def division(hh):
                            rt = small.tile(
                                [1, 512], f32, tag="rt", name=f"rt{pr}{qi}{hh}"
                            )
                            nc.vector.reciprocal(rt[0:1, :], oaccs[hh][64:65, :])
                            rb = small.tile(
                                [64, 512], f32, tag="rb", name=f"rb{pr}{qi}{hh}"
                            )
                            nc.gpsimd.partition_broadcast(
                                rb[:, :], rt[0:1, :], channels=64
                            )
                            nc.vector.tensor_tensor(
                                out=ots[pr][hh * 64 : hh * 64 + 64, qs : qs + 512],
                                in0=oaccs[hh][0:64, :],
                                in1=rb[:, :],
                                op=MUL,
                            )

                        for s in range(NKC + 4):
                            if s >= 4:  # PV for k-chunk s-4, both heads
                                kc = s - 4
                                for hh in range(2):
                                    nc.tensor.matmul(
                                        oaccs[hh][:],
                                        vts[kc][
                                            :, (2 * pr + hh) * 65 : (2 * pr + hh + 1) * 65
                                        ],
                                        pts[kc][:, hh * 512 : hh * 512 + 512],
                                        start=(kc == 0),
                                        stop=(kc == NKC - 1),
                                    )
                                if kc == NKC - 1:
                                    division(0)
                                    division(1)
                            if s < NKC:  # S for k-chunk s, both heads
                                for hh in range(2):
                                    col = (gb % 4) * 512
                                    nc.tensor.matmul(
                                        ring[:, col : col + 512],
                                        kts[pr][
                                            hh * 64 : hh * 64 + 64,
                                            s * 128 : (s + 1) * 128,
                                        ],
                                        qts[pr][hh * 64 : hh * 64 + 64, qs : qs + 512],
                                        start=True,
                                        stop=True,
                                    )
                                    gb += 1
                                base = ((gb - 2) % 4) * 512
                                pt = ppool.tile(
                                    [128, 1024], bf16, tag="p", name=f"p{pr}{qi}{s}"
                                )
                                nc.scalar.activation(
                                    pt[:],
                                    ring[:, base : base + 1024],
                                    Exp,
                                    bias=0.0,
                                    scale=SCALE,
                                )
                                pts[s] = pt
                            if pr == 0 and qi == 0 and s < NKC:
                                vfill[s]()
                            pop_filler()
                        if pr == 1:
                            queue_y(qi)

                # drain remaining fillers (tail: Y(qi3))
                while fillers:
                    fillers.pop(0)[1]()

            if iters:
                import concourse.mybir as _mb

                with tc.For_i(
                    0,
                    iters,
                    1,
                    hint_engines=(
                        _mb.EngineType.PE,
                        _mb.EngineType.Activation,
                        _mb.EngineType.DVE,
                        _mb.EngineType.SP,
                        _mb.EngineType.Pool,
                    ),
                    staggered_reset=True,
                ) as _iv:
                    body()
            else:
                body()

    nc.compile()
    return nc


def _host_inputs(x, wq_w, wq_b, wk_w, wk_b, wv_w, wv_b, wo_w, wo_b):
    """Build the 8 per-core input maps (all host-side slicing/packing)."""
    f = np.float32
    x = np.asarray(x, f)
    wq_w = np.asarray(wq_w, f)
    wk_w = np.asarray(wk_w, f)
    wv_w = np.asarray(wv_w, f)
    wo_w = np.asarray(wo_w, f)
    wq_b = np.asarray(wq_b, f)
    wk_b = np.asarray(wk_b, f)
    wv_b = np.asarray(wv_b, f)
    wo_b = np.asarray(wo_b, f)

    def chunkpack(a, ncol):  # [1024, ncol] -> [128, 8*ncol] (D-chunk packed)
        return np.ascontiguousarray(
            a.reshape(NDC, 128, ncol).transpose(1, 0, 2).reshape(128, NDC * ncol)
        )

    # RoPE tables in fp32, mirroring the reference formulas.
    pos = np.arange(T, dtype=f)[:, None]
    idx = np.arange(HALF, dtype=f)[None, :]
    inv_freq = (f(1.0) / (f(ROPE_BASE) ** (idx / f(HALF)))).astype(f)
    ang = pos * inv_freq  # [T, 32]
    cosv, sinv = np.cos(ang).astype(f), np.sin(ang).astype(f)
    cos64 = np.repeat(cosv.T, 2, axis=0)  # [64, T]
    sin64 = np.repeat(sinv.T, 2, axis=0)
    sin64[0::2] *= -1  # rows 2j: -sin, rows 2j+1: +sin
    cos128 = np.tile(cos64, (2, 1))
    sin128 = np.tile(sin64, (2, 1))

    perm64 = np.empty(64, np.int64)
    perm64[0::2] = np.arange(32)
    perm64[1::2] = np.arange(32) + 32

    xtp = [
        np.ascontiguousarray(
            x[b].T.reshape(NDC, 128, T).transpose(1, 0, 2).reshape(128, NDC * T)
        )
        for b in range(B)
    ]

    in_maps = []
    for c in range(N_CORES):
        b, g = c // 4, c % 4
        heads = np.arange(4 * g, 4 * g + 4)
        qk_cols = np.concatenate([h * 64 + perm64 for h in heads])
        v_cols = np.concatenate([np.arange(h * 64, (h + 1) * 64) for h in heads])
        wp1 = np.concatenate(
            [chunkpack(wq_w[:, qk_cols], 256), chunkpack(wk_w[:, qk_cols], 256)],
            axis=1,
        )
        wp2 = np.concatenate(
            [
                chunkpack(wv_w[:, v_cols], 256),
                cos128,
                sin128,
                wq_b[qk_cols].reshape(2, 128).T,
                wk_b[qk_cols].reshape(2, 128).T,
            ],
            axis=1,
        )
        wop = np.ascontiguousarray(
            wo_w[v_cols, :]
            .reshape(2, 128, D)
            .transpose(1, 0, 2)
            .reshape(128, 2 * D)
            .astype(BF16)
        )
        in_maps.append(
            {
                "xtp": xtp[b],
                "wp1": np.ascontiguousarray(wp1),
                "wp2": np.ascontiguousarray(wp2),
                "wop": wop,
            }
        )

    beff = (wo_b.astype(np.float64) + wv_b.astype(np.float64) @ wo_w.astype(np.float64)).astype(f)
    return in_maps, beff


def kernel(x, wq_w, wq_b, wk_w, wk_b, wv_w, wv_b, wo_w, wo_b):
    from concourse import bass2jax

    in_maps, beff = _host_inputs(
        x, wq_w, wq_b, wk_w, wk_b, wv_w, wv_b, wo_w, wo_b
    )
    if "nc" not in _ctx:
        _ctx["nc"] = _build_nc(0)
    res = bass2jax.run_bass_via_pjrt(_ctx["nc"], in_maps, n_cores=N_CORES)
    y = np.empty((B, T, D), np.float32)
    for b in range(B):
        acc = res[4 * b]["y"].copy()
        for g in range(1, 4):
            acc += res[4 * b + g]["y"]
        y[b] = acc + beff[None, :]
    return y
